# revision 30
# baseline (speedup 1.0000x reference)
"""Trainium2 Bass kernel for nn_KeypointLoss (S=3, B=8, K=11, C=23, H=W=256).

Data-parallel over batch B=8 across 8 NeuronCores: core b computes the three
losses (heatmap / label / mask) for batch element b; host assembles [B,S].

Per-core device algorithm (v6, bf16 two-tensor DVE ops + ACT casts):
  loads : 12 plane loads on the sync queue ordered [gt0,pred0,m0,mp0,
          gt1,gt2,pred1,m1,mp1,pred2,m2,mp2] (gts early so the label path
          hides in the load phase); 9 small consts on the gpsimd queue.
  casts : gt/pred/mask f32->bf16 on ACT (Copy); all heavy elementwise work
          then runs on DVE at 2x (bf16 two-tensor ops).
  heat  : pm = predb*maskb (TT bf16 2x), d = pm-gtb (TT bf16 2x in-place),
          sum d^2 per partition via STT(d,d,mult,accum) bf16.
  label : 3-level bf16 max tree for per-row max; PE transpose; one indirect
          row re-fetch per stack recovers the argmax column; the 7
          label-channel pixels per keypoint are gathered with one [77,1]
          indirect DMA per stack whose offset table is built by a tiny PE
          matmul; BCE batched on [77,3] f32.
  mask  : ACT Ln(+accum) with bf16 outputs, two small bf16 STT ops.
  final : two small matmuls reduce partition partials -> out[1,16]
"""

import numpy as np

S = 3
B = 8
K = 11
C = 23
P = 128
F = 512  # 256*256 = 128*512 plane layout
NACC = 13  # 3 heat + 3 ln1mp + 3 g*dd + 3 label + 1 heat2b col

_CACHE = {}


def _build_nc():
    import concourse.bass as bass
    import concourse.bacc as bacc
    import concourse.mybir as mybir
    import concourse.tile as tile

    dt = mybir.dt
    f32, i32, bf16 = dt.float32, dt.int32, dt.bfloat16
    Alu = mybir.AluOpType
    Act = mybir.ActivationFunctionType
    AX = mybir.AxisListType.X

    nc = bacc.Bacc("TRN2", target_bir_lowering=False, debug=False)
    cp = nc.declare_dram_parameter("cp", [S, C, P, F], f32, isOutput=False)
    hm = nc.declare_dram_parameter("hm", [S, K, P, F], f32, isOutput=False)
    mk = nc.declare_dram_parameter("mk", [S, P, F], f32, isOutput=False)
    lab = nc.declare_dram_parameter("lab", [K, 7], f32, isOutput=False)
    wmp = nc.declare_dram_parameter("wm", [NACC, 9], f32, isOutput=False)
    idp = nc.declare_dram_parameter("identb", [128, 128], bf16, isOutput=False)
    iop = nc.declare_dram_parameter("iotap", [K, 128], f32, isOutput=False)
    iofp = nc.declare_dram_parameter("iotaf128", [128, F], f32, isOutput=False)
    skp = nc.declare_dram_parameter("sk113", [K, 3], f32, isOutput=False)
    m77p = nc.declare_dram_parameter("m77k", [K, 77], f32, isOutput=False)
    cvp = nc.declare_dram_parameter("cvecs77", [77, 3], f32, isOutput=False)
    selp = nc.declare_dram_parameter("sel77", [77, K], f32, isOutput=False)
    out = nc.declare_dram_parameter("out", [1, 16], f32, isOutput=True)

    hm_flat = hm[:].rearrange("s k p f -> (s k p) f")     # 512-wide rows
    cp_pix = cp[:].rearrange("s c p (f one) -> (s c p f) one", one=1)
    lab77v = lab[:].rearrange("k (c one) -> (k c) one", one=1)

    with tile.TileContext(nc) as tc:
        with (
            tc.tile_pool(name="const", bufs=1) as cst,
            tc.tile_pool(name="accp", bufs=1) as accp,
            tc.tile_pool(name="big", bufs=2) as big,
            tc.tile_pool(name="sm", bufs=1) as sm,
            tc.tile_pool(name="ps", bufs=2, space="PSUM") as ps,
        ):
            # --------- constants (gpsimd queue, small) + memsets ---------
            identb = cst.tile([128, 128], bf16)
            nc.scalar.dma_start(out=identb[:], in_=idp[:])
            iotaP = cst.tile([K, 128], f32)
            nc.scalar.dma_start(out=iotaP[:], in_=iop[:])
            iotaF128 = cst.tile([128, F], f32)
            nc.scalar.dma_start(out=iotaF128[:], in_=iofp[:])
            sk113 = cst.tile([K, 3], f32)
            nc.scalar.dma_start(out=sk113[:], in_=skp[:])
            m77k = cst.tile([K, 77], f32)
            nc.scalar.dma_start(out=m77k[:], in_=m77p[:])
            cvecs77 = cst.tile([77, 3], f32)
            nc.scalar.dma_start(out=cvecs77[:], in_=cvp[:])
            sel77 = cst.tile([77, K], f32)
            nc.scalar.dma_start(out=sel77[:], in_=selp[:])
            lab77 = cst.tile([77, 1], f32)
            nc.scalar.dma_start(out=lab77[:], in_=lab77v)
            Wm = cst.tile([NACC, 9], f32)
            nc.scalar.dma_start(out=Wm[:], in_=wmp[:])
            ones = cst.tile([128, 1], f32)
            nc.vector.memset(ones[:], 1.0)

            acc = accp.tile([128, NACC], f32)
            nc.vector.memset(acc[:], 0.0)

            # ------------- input loads: one ordered sync queue -------------
            gts = [None] * S
            preds = [None] * S
            masks = [None] * S
            mpreds = [None] * S

            def load_gt(s):
                gts[s] = big.tile([P, K, F], f32, tag="gt", bufs=2, name=f"gt{s}")
                nc.sync.dma_start(out=gts[s][:],
                                  in_=hm[s].rearrange("k p f -> p k f"))

            def load_pred(s):
                preds[s] = big.tile([P, K, F], f32, tag="pred", name=f"pred{s}")
                nc.sync.dma_start(out=preds[s][:],
                                  in_=cp[s, K:2 * K].rearrange("k p f -> p k f"))

            def load_mm(s):
                masks[s] = big.tile([P, F], f32, tag="mask", bufs=2, name=f"mask{s}")
                mpreds[s] = big.tile([P, F], f32, tag="mpred", bufs=2, name=f"mpred{s}")
                nc.sync.dma_start(out=masks[s][:], in_=mk[s])
                nc.sync.dma_start(out=mpreds[s][:], in_=cp[s, 2 * K])

            load_gt(0); load_pred(0); load_mm(0)
            load_gt(1); load_gt(2)
            load_pred(1); load_mm(1)
            load_pred(2); load_mm(2)

            # chain-A result tiles shared across stacks (column s each)
            Mx113 = sm.tile([K, 3], f32)
            pstar113 = sm.tile([K, 3], f32)
            pstar_c = sm.tile([K, 3], f32)
            idxg_i = sm.tile([K, 3], i32)
            grow3 = sm.tile([K, 3, F], f32)
            fstar113 = sm.tile([K, 3], f32)
            fstar_c = sm.tile([K, 3], f32)
            fidx113 = sm.tile([K, 3], f32)
            ps_idx = ps.tile([77, 3], f32, tag="psidx", bufs=1)
            idx77i = sm.tile([77, 3], i32)
            G77 = sm.tile([77, 3], f32)
            iotaF11 = iotaF128[0:K, :]

            gtbs, predbs, maskbs, pms = [None] * S, [None] * S, [None] * S, [None] * S

            def cast_gt(s):
                gtbs[s] = big.tile([P, K, F], bf16, tag="gtb", name=f"gtb{s}")
                nc.scalar.activation(out=gtbs[s][:], in_=gts[s][:], func=Act.Copy)

            def cast_pm(s):
                predbs[s] = big.tile([P, K, F], bf16, tag="predb", name=f"predb{s}")
                if s <= 2:
                    nc.scalar.activation(out=predbs[s][:], in_=preds[s][:], func=Act.Copy)
                else:
                    nc.vector.tensor_copy(predbs[s][:], preds[s][:])
                maskbs[s] = big.tile([P, F], bf16, tag="maskb", bufs=3, name=f"maskb{s}")
                nc.vector.tensor_copy(maskbs[s][:], masks[s][:])

            def heat(s):
                # pm = predb*maskb (2x), d = pm-gtb (2x, in-place)
                pm = big.tile([P, K, F], bf16, tag="pm", name=f"pm{s}")
                mask_b = maskbs[s][:].rearrange("p (a f) -> p a f", a=1).to_broadcast([P, K, F])
                nc.vector.tensor_tensor(out=pm[:], in0=predbs[s][:], in1=mask_b, op=Alu.mult)
                nc.vector.tensor_tensor(out=pm[:], in0=pm[:], in1=gtbs[s][:], op=Alu.subtract)
                pms[s] = pm

            def square(s):
                if True:
                    nc.scalar.activation(out=pms[s][:], in_=pms[s][:], func=Act.Square,
                                         accum_out=acc[:, s:s + 1])
                else:
                    # tail stack: split across ACT and DVE (partials via Wm)
                    h = (K * F) // 2
                    pmf = pms[s][:].rearrange("p k f -> p (k f)")
                    nc.scalar.activation(out=pmf[:, 0:h], in_=pmf[:, 0:h],
                                         func=Act.Square, accum_out=acc[:, s:s + 1])
                    nc.vector.scalar_tensor_tensor(out=pmf[:, h:2 * h], in0=pmf[:, h:2 * h],
                                                   scalar=0.0, in1=pmf[:, h:2 * h],
                                                   op0=Alu.bypass, op1=Alu.mult,
                                                   accum_out=acc[:, 12:13])

            def tree_chain(s):
                # rowmax: 3-level bf16 max tree
                gtb = gtbs[s]
                h1 = big.tile([P, K, 256], bf16, tag="h1", bufs=2, name=f"h1_{s}")
                nc.vector.tensor_tensor(out=h1[:], in0=gtb[:, :, 0:256],
                                        in1=gtb[:, :, 256:512], op=Alu.max)
                nc.vector.tensor_tensor(out=h1[:, :, 0:128], in0=h1[:, :, 0:128],
                                        in1=h1[:, :, 128:256], op=Alu.max)
                nc.vector.tensor_tensor(out=h1[:, :, 0:64], in0=h1[:, :, 0:64],
                                        in1=h1[:, :, 64:128], op=Alu.max)
                rowmax = sm.tile([P, K], bf16, tag="rowmax", bufs=3)
                nc.vector.tensor_reduce(out=rowmax[:], in_=h1[:, :, 0:64],
                                        axis=AX, op=Alu.max)
                pt = ps.tile([K, 128], bf16, tag="pt", bufs=2)
                nc.tensor.transpose(out=pt[:], in_=rowmax[:], identity=identb[:])
                nc.vector.tensor_reduce(out=Mx113[:, s:s + 1], in_=pt[:], axis=AX, op=Alu.max)
                oh = sm.tile([K, 128], f32, tag="oh", bufs=2)
                nc.vector.tensor_scalar(out=oh[:], in0=pt[:], scalar1=Mx113[:, s:s + 1],
                                        scalar2=None, op0=Alu.is_equal)
                scrP = sm.tile([K, 128], f32, tag="scrP", bufs=2)
                nc.vector.scalar_tensor_tensor(out=scrP[:], in0=oh[:], scalar=0.0,
                                               in1=iotaP[:], op0=Alu.bypass, op1=Alu.mult,
                                               accum_out=pstar113[:, s:s + 1])
                nc.vector.tensor_scalar(out=pstar_c[:, s:s + 1], in0=pstar113[:, s:s + 1],
                                        scalar1=0.0, scalar2=127.0,
                                        op0=Alu.max, op1=Alu.min)
                nc.vector.tensor_tensor(out=idxg_i[:, s:s + 1], in0=pstar_c[:, s:s + 1],
                                        in1=sk113[:, s:s + 1], op=Alu.add)
                nc.gpsimd.indirect_dma_start(
                    out=grow3[:, s, :], out_offset=None, in_=hm_flat,
                    in_offset=bass.IndirectOffsetOnAxis(ap=idxg_i[:, s:s + 1], axis=0))

            def wsel_g77(s):
                # runs >=1 phase after grow_s was issued (hides gather latency)
                nc.vector.scalar_tensor_tensor(out=grow3[:, s, :], in0=grow3[:, s, :],
                                               scalar=Mx113[:, s:s + 1], in1=iotaF11,
                                               op0=Alu.is_equal, op1=Alu.mult,
                                               accum_out=fstar113[:, s:s + 1])
                nc.vector.tensor_scalar(out=fstar_c[:, s:s + 1], in0=fstar113[:, s:s + 1],
                                        scalar1=0.0, scalar2=511.0,
                                        op0=Alu.max, op1=Alu.min)
                nc.vector.scalar_tensor_tensor(out=fidx113[:, s:s + 1],
                                               in0=pstar_c[:, s:s + 1], scalar=512.0,
                                               in1=fstar_c[:, s:s + 1],
                                               op0=Alu.mult, op1=Alu.add)
                nc.tensor.matmul(out=ps_idx[:, s:s + 1], lhsT=m77k[:],
                                 rhs=fidx113[:, s:s + 1], start=True, stop=True)
                nc.vector.tensor_tensor(out=idx77i[:, s:s + 1], in0=ps_idx[:, s:s + 1],
                                        in1=cvecs77[:, s:s + 1], op=Alu.add)
                nc.gpsimd.indirect_dma_start(
                    out=G77[:, s:s + 1], out_offset=None, in_=cp_pix,
                    in_offset=bass.IndirectOffsetOnAxis(ap=idx77i[:, s:s + 1], axis=0))

            def mask_loss(s):
                ln1m = big.tile([P, F], bf16, tag="ln1m", name=f"ln1m{s}")
                lnp = big.tile([P, F], bf16, tag="lnp", name=f"lnp{s}")
                nc.scalar.activation(out=ln1m[:], in_=mpreds[s][:], func=Act.Ln,
                                     bias=1.0, scale=-1.0, accum_out=acc[:, 3 + s:4 + s])
                nc.scalar.activation(out=lnp[:], in_=mpreds[s][:], func=Act.Ln)
                nc.vector.tensor_tensor(out=lnp[:], in0=lnp[:], in1=ln1m[:],
                                        op=Alu.subtract)
                nc.vector.scalar_tensor_tensor(out=lnp[:], in0=lnp[:], scalar=0.0,
                                               in1=maskbs[s][:], op0=Alu.bypass,
                                               op1=Alu.mult,
                                               accum_out=acc[:, 6 + s:7 + s])

            # explicit phase schedule: stack2's gt path is hoisted so every
            # gather completes mid-pipeline; squares trail by one stack
            cast_gt(0); cast_pm(0)
            heat(0)
            tree_chain(0)
            mask_loss(0)
            cast_gt(1); cast_gt(2)
            tree_chain(1)
            wsel_g77(0)
            cast_pm(1)
            heat(1)
            tree_chain(2)
            wsel_g77(1)
            cast_pm(2)
            square(0)
            heat(2)
            square(1)
            wsel_g77(2)
            mask_loss(1)
            mask_loss(2)
            square(2)

            # ------------- label BCE (all stacks) -------------
            valid113 = sm.tile([K, 3], f32)
            nc.vector.tensor_scalar(out=valid113[:], in0=Mx113[:], scalar1=1.0,
                                    scalar2=None, op0=Alu.is_equal)
            ln1m77 = sm.tile([77, 3], f32)
            lnp77 = sm.tile([77, 3], f32)
            nc.scalar.activation(out=ln1m77[:], in_=G77[:], func=Act.Ln,
                                 bias=1.0, scale=-1.0)
            nc.scalar.activation(out=lnp77[:], in_=G77[:], func=Act.Ln)
            dd77 = sm.tile([77, 3], f32)
            nc.vector.tensor_tensor(out=dd77[:], in0=lnp77[:], in1=ln1m77[:], op=Alu.subtract)
            nc.vector.tensor_scalar(out=dd77[:], in0=dd77[:], scalar1=lab77[:, 0:1],
                                    scalar2=None, op0=Alu.mult)
            bce77 = sm.tile([77, 3], f32)
            nc.vector.tensor_tensor(out=bce77[:], in0=dd77[:], in1=ln1m77[:], op=Alu.add)
            lbl_ps = ps.tile([K, 3], f32, tag="lblps", bufs=1)
            nc.tensor.matmul(out=lbl_ps[:], lhsT=sel77[:], rhs=bce77[:], start=True, stop=True)
            for s in range(S):
                nc.vector.tensor_tensor(out=acc[0:K, 9 + s:10 + s],
                                        in0=lbl_ps[:, s:s + 1],
                                        in1=valid113[:, s:s + 1], op=Alu.mult)

            # ------------- final reduction -------------
            acc2 = accp.tile([128, NACC], f32)
            nc.vector.tensor_copy(acc2[:], acc[:])
            ps1 = ps.tile([NACC, 1], f32, tag="ps1", bufs=1)
            nc.tensor.matmul(out=ps1[:], lhsT=acc2[:], rhs=ones[:], start=True, stop=True)
            s1 = sm.tile([NACC, 1], f32)
            nc.vector.tensor_copy(s1[:], ps1[:])
            ps2 = ps.tile([1, 9], f32, tag="ps2", bufs=1)
            nc.tensor.matmul(out=ps2[:], lhsT=s1[:], rhs=Wm[:], start=True, stop=True)
            res = sm.tile([1, 16], f32)
            nc.vector.memset(res[:], 0.0)
            nc.vector.tensor_copy(res[0:1, 0:9], ps2[:])
            nc.sync.dma_start(out=out[:], in_=res[:])

    nc.finalize()
    return nc


def get_nc():
    if "nc" not in _CACHE:
        _CACHE["nc"] = _build_nc()
    return _CACHE["nc"]


def _make_wm():
    wm = np.zeros((NACC, 9), dtype=np.float32)
    for s in range(S):
        wm[s, s] = 1.0 / 11.0                # heat: accum is sum over K,pix
        wm[3 + s, 3 + s] = -1.0 / 65536.0    # mask: -(A+B)/HW
        wm[6 + s, 3 + s] = -1.0 / 65536.0
        wm[9 + s, 6 + s] = -1.0 / 77.0       # label: -sum/(7*11)
    wm[12, S - 1] = 1.0 / 11.0               # heat stack2: DVE-half partial
    return wm


def _consts():
    if "consts" in _CACHE:
        return _CACHE["consts"]
    import ml_dtypes
    identb = np.eye(128, dtype=np.float32).astype(ml_dtypes.bfloat16)
    iotap = np.broadcast_to(np.arange(128, dtype=np.float32), (K, 128)).copy()
    iotaf128 = np.broadcast_to(np.arange(F, dtype=np.float32), (128, F)).copy()
    ks = np.arange(K, dtype=np.float32)[:, None] * 128.0
    ss = np.arange(S, dtype=np.float32)[None, :] * (K * 128.0)
    sk113 = (ks + ss).astype(np.float32)                      # [K,3]
    r = np.arange(77)
    m77k = np.zeros((K, 77), dtype=np.float32)
    m77k[r // 7, r] = 1.0                                     # [K,77] lhsT
    cvecs77 = ((r % 7)[:, None] * 65536.0 +
               np.arange(S)[None, :] * (C * 65536.0)).astype(np.float32)  # [77,3]
    sel77 = np.zeros((77, K), dtype=np.float32)
    sel77[r, r // 7] = 1.0                                    # [77,K] lhsT
    _CACHE["consts"] = dict(wm=_make_wm(), identb=identb, iotap=iotap,
                            iotaf128=iotaf128, sk113=sk113, m77k=m77k,
                            cvecs77=cvecs77, sel77=sel77)
    return _CACHE["consts"]


def make_in_maps(combined_preds, heatmaps, labels, masks):
    cpn = np.asarray(combined_preds, dtype=np.float32)
    hmn = np.asarray(heatmaps, dtype=np.float32)
    lbn = np.asarray(labels, dtype=np.float32)
    mkn = np.asarray(masks, dtype=np.float32)
    cc = _consts()
    in_maps = []
    for b in range(B):
        m = {
            "cp": np.ascontiguousarray(cpn[:, b]).reshape(S, C, P, F),
            "hm": np.ascontiguousarray(hmn[:, b]).reshape(S, K, P, F),
            "mk": np.ascontiguousarray(mkn[:, b, 0]).reshape(S, P, F),
            "lab": np.ascontiguousarray(lbn[b]),
        }
        m.update(cc)
        in_maps.append(m)
    return in_maps


def run_spmd(in_maps, trace=False, **kw):
    from concourse.bass_utils import run_bass_kernel_spmd
    return run_bass_kernel_spmd(get_nc(), in_maps, core_ids=list(range(B)),
                                trace=trace, **kw)


def kernel(combined_preds, heatmaps, labels, masks):
    res = run_spmd(make_in_maps(combined_preds, heatmaps, labels, masks)).results
    heat = np.stack([res[b]["out"][0, 0:3] for b in range(B)]).astype(np.float32)
    mask_l = np.stack([res[b]["out"][0, 3:6] for b in range(B)]).astype(np.float32)
    label = np.stack([res[b]["out"][0, 6:9] for b in range(B)]).astype(np.float32)
    return (heat, label, mask_l)


# revision 31
# speedup vs baseline: 1.0647x; 1.0647x over previous
"""Trainium2 Bass kernel for nn_KeypointLoss (S=3, B=8, K=11, C=23, H=W=256).

Data-parallel over batch B=8 across 8 NeuronCores: core b computes the three
losses (heatmap / label / mask) for batch element b; host assembles [B,S].

Per-core device algorithm (final: bf16 DVE two-tensor ops, ACT casts+squares):
  loads : 12 plane loads on the sync queue ordered [gt0,pred0,m0,mp0,
          gt1,gt2,pred1,m1,mp1,pred2,m2,mp2] (gts early so the whole label
          path hides inside the load phase); 9 small consts on scalar queue.
  casts : gt/pred/mask f32->bf16 all on ACT (keeps DVE lean; DVE two-tensor
          bf16 ops then run at 2 elem/cycle).
  heat  : pm = predb*maskb (TT bf16 2x), d = pm-gtb (TT bf16 2x in-place),
          Square+accum per stack on ACT, deferred one stack so it never
          blocks the next stack's casts.
  label : 4-level bf16 max tree for per-row max; PE transpose; one indirect
          row re-fetch per stack recovers the argmax column (clamped indices
          keep masked-invalid maps in bounds); the 7 label-channel pixels
          per keypoint are gathered with one [77,1] indirect DMA per stack
          whose offset table is built by a tiny PE matmul; BCE on [77,3].
  mask  : ACT Ln(+accum) with bf16 outputs, two small DVE STT ops.
  final : two small matmuls reduce partition partials -> out[1,16]
"""

import numpy as np

S = 3
B = 8
K = 11
C = 23
P = 128
F = 512  # 256*256 = 128*512 plane layout
NACC = 13  # 3 heat + 3 ln1mp + 3 g*dd + 3 label + 1 heat2b col

_CACHE = {}


def _build_nc():
    import concourse.bass as bass
    import concourse.bacc as bacc
    import concourse.mybir as mybir
    import concourse.tile as tile

    dt = mybir.dt
    f32, i32, bf16 = dt.float32, dt.int32, dt.bfloat16
    Alu = mybir.AluOpType
    Act = mybir.ActivationFunctionType
    AX = mybir.AxisListType.X

    nc = bacc.Bacc("TRN2", target_bir_lowering=False, debug=False)
    cp = nc.declare_dram_parameter("cp", [S, C, P, F], f32, isOutput=False)
    hm = nc.declare_dram_parameter("hm", [S, K, P, F], f32, isOutput=False)
    mk = nc.declare_dram_parameter("mk", [S, P, F], f32, isOutput=False)
    lab = nc.declare_dram_parameter("lab", [K, 7], f32, isOutput=False)
    wmp = nc.declare_dram_parameter("wm", [NACC, 9], f32, isOutput=False)
    idp = nc.declare_dram_parameter("identb", [128, 128], bf16, isOutput=False)
    iop = nc.declare_dram_parameter("iotap", [K, 128], f32, isOutput=False)
    iofp = nc.declare_dram_parameter("iotaf128", [128, F], f32, isOutput=False)
    skp = nc.declare_dram_parameter("sk113", [K, 3], f32, isOutput=False)
    m77p = nc.declare_dram_parameter("m77k", [K, 77], f32, isOutput=False)
    cvp = nc.declare_dram_parameter("cvecs77", [77, 3], f32, isOutput=False)
    selp = nc.declare_dram_parameter("sel77", [77, K], f32, isOutput=False)
    out = nc.declare_dram_parameter("out", [1, 16], f32, isOutput=True)

    hm_flat = hm[:].rearrange("s k p f -> (s k p) f")     # 512-wide rows
    cp_pix = cp[:].rearrange("s c p (f one) -> (s c p f) one", one=1)
    lab77v = lab[:].rearrange("k (c one) -> (k c) one", one=1)

    with tile.TileContext(nc) as tc:
        with (
            tc.tile_pool(name="const", bufs=1) as cst,
            tc.tile_pool(name="accp", bufs=1) as accp,
            tc.tile_pool(name="big", bufs=2) as big,
            tc.tile_pool(name="sm", bufs=1) as sm,
            tc.tile_pool(name="ps", bufs=2, space="PSUM") as ps,
        ):
            # --------- constants (gpsimd queue, small) + memsets ---------
            identb = cst.tile([128, 128], bf16)
            nc.scalar.dma_start(out=identb[:], in_=idp[:])
            iotaP = cst.tile([K, 128], f32)
            nc.scalar.dma_start(out=iotaP[:], in_=iop[:])
            iotaF128 = cst.tile([128, F], f32)
            nc.scalar.dma_start(out=iotaF128[:], in_=iofp[:])
            sk113 = cst.tile([K, 3], f32)
            nc.scalar.dma_start(out=sk113[:], in_=skp[:])
            m77k = cst.tile([K, 77], f32)
            nc.scalar.dma_start(out=m77k[:], in_=m77p[:])
            cvecs77 = cst.tile([77, 3], f32)
            nc.scalar.dma_start(out=cvecs77[:], in_=cvp[:])
            sel77 = cst.tile([77, K], f32)
            nc.scalar.dma_start(out=sel77[:], in_=selp[:])
            lab77 = cst.tile([77, 1], f32)
            nc.scalar.dma_start(out=lab77[:], in_=lab77v)
            Wm = cst.tile([NACC, 9], f32)
            nc.scalar.dma_start(out=Wm[:], in_=wmp[:])
            ones = cst.tile([128, 1], f32)
            nc.vector.memset(ones[:], 1.0)

            acc = accp.tile([128, NACC], f32)
            nc.vector.memset(acc[:], 0.0)

            # ------------- input loads: one ordered sync queue -------------
            gts = [None] * S
            preds = [None] * S
            masks = [None] * S
            mpreds = [None] * S

            def load_gt(s):
                gts[s] = big.tile([P, K, F], f32, tag="gt", bufs=2, name=f"gt{s}")
                nc.sync.dma_start(out=gts[s][:],
                                  in_=hm[s].rearrange("k p f -> p k f"))

            def load_pred(s):
                preds[s] = big.tile([P, K, F], f32, tag="pred", name=f"pred{s}")
                nc.sync.dma_start(out=preds[s][:],
                                  in_=cp[s, K:2 * K].rearrange("k p f -> p k f"))

            def load_mm(s):
                masks[s] = big.tile([P, F], f32, tag="mask", bufs=2, name=f"mask{s}")
                mpreds[s] = big.tile([P, F], f32, tag="mpred", bufs=2, name=f"mpred{s}")
                nc.sync.dma_start(out=masks[s][:], in_=mk[s])
                nc.sync.dma_start(out=mpreds[s][:], in_=cp[s, 2 * K])

            load_gt(0); load_pred(0); load_mm(0)
            load_gt(1); load_gt(2)
            load_pred(1); load_mm(1)
            load_pred(2); load_mm(2)

            # chain-A result tiles shared across stacks (column s each)
            Mx113 = sm.tile([K, 3], f32)
            pstar113 = sm.tile([K, 3], f32)
            pstar_c = sm.tile([K, 3], f32)
            idxg_i = sm.tile([K, 3], i32)
            grow3 = sm.tile([K, 3, F], f32)
            fstar113 = sm.tile([K, 3], f32)
            fstar_c = sm.tile([K, 3], f32)
            fidx113 = sm.tile([K, 3], f32)
            ps_idx = ps.tile([77, 3], f32, tag="psidx", bufs=1)
            idx77i = sm.tile([77, 3], i32)
            G77 = sm.tile([77, 3], f32)
            iotaF11 = iotaF128[0:K, :]

            gtbs, predbs, maskbs, pms = [None] * S, [None] * S, [None] * S, [None] * S

            def cast_gt(s):
                gtbs[s] = big.tile([P, K, F], bf16, tag="gtb", name=f"gtb{s}")
                nc.scalar.activation(out=gtbs[s][:], in_=gts[s][:], func=Act.Copy)

            def cast_pm(s):
                predbs[s] = big.tile([P, K, F], bf16, tag="predb", name=f"predb{s}")
                if s <= 2:
                    nc.scalar.activation(out=predbs[s][:], in_=preds[s][:], func=Act.Copy)
                else:
                    nc.vector.tensor_copy(predbs[s][:], preds[s][:])
                maskbs[s] = big.tile([P, F], bf16, tag="maskb", bufs=3, name=f"maskb{s}")
                nc.scalar.activation(out=maskbs[s][:], in_=masks[s][:], func=Act.Copy)

            def heat(s):
                # pm = predb*maskb (2x), d = pm-gtb (2x, in-place)
                pm = big.tile([P, K, F], bf16, tag="pm", name=f"pm{s}")
                mask_b = maskbs[s][:].rearrange("p (a f) -> p a f", a=1).to_broadcast([P, K, F])
                nc.vector.tensor_tensor(out=pm[:], in0=predbs[s][:], in1=mask_b, op=Alu.mult)
                nc.vector.tensor_tensor(out=pm[:], in0=pm[:], in1=gtbs[s][:], op=Alu.subtract)
                pms[s] = pm

            def square(s):
                if True:
                    nc.scalar.activation(out=pms[s][:], in_=pms[s][:], func=Act.Square,
                                         accum_out=acc[:, s:s + 1])
                else:
                    # tail stack: split across ACT and DVE (partials via Wm)
                    h = (K * F) // 2
                    pmf = pms[s][:].rearrange("p k f -> p (k f)")
                    nc.scalar.activation(out=pmf[:, 0:h], in_=pmf[:, 0:h],
                                         func=Act.Square, accum_out=acc[:, s:s + 1])
                    nc.vector.scalar_tensor_tensor(out=pmf[:, h:2 * h], in0=pmf[:, h:2 * h],
                                                   scalar=0.0, in1=pmf[:, h:2 * h],
                                                   op0=Alu.bypass, op1=Alu.mult,
                                                   accum_out=acc[:, 12:13])

            def tree_chain(s):
                # rowmax: 3-level bf16 max tree
                gtb = gtbs[s]
                h1 = big.tile([P, K, 256], bf16, tag="h1", bufs=2, name=f"h1_{s}")
                nc.vector.tensor_tensor(out=h1[:], in0=gtb[:, :, 0:256],
                                        in1=gtb[:, :, 256:512], op=Alu.max)
                nc.vector.tensor_tensor(out=h1[:, :, 0:128], in0=h1[:, :, 0:128],
                                        in1=h1[:, :, 128:256], op=Alu.max)
                nc.vector.tensor_tensor(out=h1[:, :, 0:64], in0=h1[:, :, 0:64],
                                        in1=h1[:, :, 64:128], op=Alu.max)
                rowmax = sm.tile([P, K], bf16, tag="rowmax", bufs=3)
                nc.vector.tensor_reduce(out=rowmax[:], in_=h1[:, :, 0:64],
                                        axis=AX, op=Alu.max)
                pt = ps.tile([K, 128], bf16, tag="pt", bufs=2)
                nc.tensor.transpose(out=pt[:], in_=rowmax[:], identity=identb[:])
                nc.vector.tensor_reduce(out=Mx113[:, s:s + 1], in_=pt[:], axis=AX, op=Alu.max)
                oh = sm.tile([K, 128], f32, tag="oh", bufs=2)
                nc.vector.tensor_scalar(out=oh[:], in0=pt[:], scalar1=Mx113[:, s:s + 1],
                                        scalar2=None, op0=Alu.is_equal)
                scrP = sm.tile([K, 128], f32, tag="scrP", bufs=2)
                nc.vector.scalar_tensor_tensor(out=scrP[:], in0=oh[:], scalar=0.0,
                                               in1=iotaP[:], op0=Alu.bypass, op1=Alu.mult,
                                               accum_out=pstar113[:, s:s + 1])
                nc.vector.tensor_scalar(out=pstar_c[:, s:s + 1], in0=pstar113[:, s:s + 1],
                                        scalar1=0.0, scalar2=127.0,
                                        op0=Alu.max, op1=Alu.min)
                nc.vector.tensor_tensor(out=idxg_i[:, s:s + 1], in0=pstar_c[:, s:s + 1],
                                        in1=sk113[:, s:s + 1], op=Alu.add)
                nc.gpsimd.indirect_dma_start(
                    out=grow3[:, s, :], out_offset=None, in_=hm_flat,
                    in_offset=bass.IndirectOffsetOnAxis(ap=idxg_i[:, s:s + 1], axis=0))

            def wsel_g77(s):
                # runs >=1 phase after grow_s was issued (hides gather latency)
                nc.vector.scalar_tensor_tensor(out=grow3[:, s, :], in0=grow3[:, s, :],
                                               scalar=Mx113[:, s:s + 1], in1=iotaF11,
                                               op0=Alu.is_equal, op1=Alu.mult,
                                               accum_out=fstar113[:, s:s + 1])
                nc.vector.tensor_scalar(out=fstar_c[:, s:s + 1], in0=fstar113[:, s:s + 1],
                                        scalar1=0.0, scalar2=511.0,
                                        op0=Alu.max, op1=Alu.min)
                nc.vector.scalar_tensor_tensor(out=fidx113[:, s:s + 1],
                                               in0=pstar_c[:, s:s + 1], scalar=512.0,
                                               in1=fstar_c[:, s:s + 1],
                                               op0=Alu.mult, op1=Alu.add)
                nc.tensor.matmul(out=ps_idx[:, s:s + 1], lhsT=m77k[:],
                                 rhs=fidx113[:, s:s + 1], start=True, stop=True)
                nc.vector.tensor_tensor(out=idx77i[:, s:s + 1], in0=ps_idx[:, s:s + 1],
                                        in1=cvecs77[:, s:s + 1], op=Alu.add)
                nc.gpsimd.indirect_dma_start(
                    out=G77[:, s:s + 1], out_offset=None, in_=cp_pix,
                    in_offset=bass.IndirectOffsetOnAxis(ap=idx77i[:, s:s + 1], axis=0))

            def mask_loss(s):
                ln1m = big.tile([P, F], bf16, tag="ln1m", name=f"ln1m{s}")
                lnp = big.tile([P, F], bf16, tag="lnp", name=f"lnp{s}")
                nc.scalar.activation(out=ln1m[:], in_=mpreds[s][:], func=Act.Ln,
                                     bias=1.0, scale=-1.0, accum_out=acc[:, 3 + s:4 + s])
                nc.scalar.activation(out=lnp[:], in_=mpreds[s][:], func=Act.Ln)
                nc.vector.tensor_tensor(out=lnp[:], in0=lnp[:], in1=ln1m[:],
                                        op=Alu.subtract)
                nc.vector.scalar_tensor_tensor(out=lnp[:], in0=lnp[:], scalar=0.0,
                                               in1=maskbs[s][:], op0=Alu.bypass,
                                               op1=Alu.mult,
                                               accum_out=acc[:, 6 + s:7 + s])

            # explicit phase schedule: stack2's gt path is hoisted so every
            # gather completes mid-pipeline; squares trail by one stack
            cast_gt(0); cast_pm(0)
            heat(0)
            tree_chain(0)
            mask_loss(0)
            cast_gt(1); cast_gt(2)
            tree_chain(1)
            wsel_g77(0)
            cast_pm(1)
            heat(1)
            tree_chain(2)
            wsel_g77(1)
            cast_pm(2)
            square(0)
            heat(2)
            square(1)
            wsel_g77(2)
            mask_loss(1)
            mask_loss(2)
            square(2)

            # ------------- label BCE (all stacks) -------------
            valid113 = sm.tile([K, 3], f32)
            nc.vector.tensor_scalar(out=valid113[:], in0=Mx113[:], scalar1=1.0,
                                    scalar2=None, op0=Alu.is_equal)
            ln1m77 = sm.tile([77, 3], f32)
            lnp77 = sm.tile([77, 3], f32)
            nc.scalar.activation(out=ln1m77[:], in_=G77[:], func=Act.Ln,
                                 bias=1.0, scale=-1.0)
            nc.scalar.activation(out=lnp77[:], in_=G77[:], func=Act.Ln)
            dd77 = sm.tile([77, 3], f32)
            nc.vector.tensor_tensor(out=dd77[:], in0=lnp77[:], in1=ln1m77[:], op=Alu.subtract)
            nc.vector.tensor_scalar(out=dd77[:], in0=dd77[:], scalar1=lab77[:, 0:1],
                                    scalar2=None, op0=Alu.mult)
            bce77 = sm.tile([77, 3], f32)
            nc.vector.tensor_tensor(out=bce77[:], in0=dd77[:], in1=ln1m77[:], op=Alu.add)
            lbl_ps = ps.tile([K, 3], f32, tag="lblps", bufs=1)
            nc.tensor.matmul(out=lbl_ps[:], lhsT=sel77[:], rhs=bce77[:], start=True, stop=True)
            for s in range(S):
                nc.vector.tensor_tensor(out=acc[0:K, 9 + s:10 + s],
                                        in0=lbl_ps[:, s:s + 1],
                                        in1=valid113[:, s:s + 1], op=Alu.mult)

            # ------------- final reduction -------------
            acc2 = accp.tile([128, NACC], f32)
            nc.vector.tensor_copy(acc2[:], acc[:])
            ps1 = ps.tile([NACC, 1], f32, tag="ps1", bufs=1)
            nc.tensor.matmul(out=ps1[:], lhsT=acc2[:], rhs=ones[:], start=True, stop=True)
            s1 = sm.tile([NACC, 1], f32)
            nc.vector.tensor_copy(s1[:], ps1[:])
            ps2 = ps.tile([1, 9], f32, tag="ps2", bufs=1)
            nc.tensor.matmul(out=ps2[:], lhsT=s1[:], rhs=Wm[:], start=True, stop=True)
            res = sm.tile([1, 16], f32)
            nc.vector.memset(res[:], 0.0)
            nc.vector.tensor_copy(res[0:1, 0:9], ps2[:])
            nc.sync.dma_start(out=out[:], in_=res[:])

    nc.finalize()
    return nc


def get_nc():
    if "nc" not in _CACHE:
        _CACHE["nc"] = _build_nc()
    return _CACHE["nc"]


def _make_wm():
    wm = np.zeros((NACC, 9), dtype=np.float32)
    for s in range(S):
        wm[s, s] = 1.0 / 11.0                # heat: accum is sum over K,pix
        wm[3 + s, 3 + s] = -1.0 / 65536.0    # mask: -(A+B)/HW
        wm[6 + s, 3 + s] = -1.0 / 65536.0
        wm[9 + s, 6 + s] = -1.0 / 77.0       # label: -sum/(7*11)
    wm[12, S - 1] = 1.0 / 11.0               # heat stack2: DVE-half partial
    return wm


def _consts():
    if "consts" in _CACHE:
        return _CACHE["consts"]
    import ml_dtypes
    identb = np.eye(128, dtype=np.float32).astype(ml_dtypes.bfloat16)
    iotap = np.broadcast_to(np.arange(128, dtype=np.float32), (K, 128)).copy()
    iotaf128 = np.broadcast_to(np.arange(F, dtype=np.float32), (128, F)).copy()
    ks = np.arange(K, dtype=np.float32)[:, None] * 128.0
    ss = np.arange(S, dtype=np.float32)[None, :] * (K * 128.0)
    sk113 = (ks + ss).astype(np.float32)                      # [K,3]
    r = np.arange(77)
    m77k = np.zeros((K, 77), dtype=np.float32)
    m77k[r // 7, r] = 1.0                                     # [K,77] lhsT
    cvecs77 = ((r % 7)[:, None] * 65536.0 +
               np.arange(S)[None, :] * (C * 65536.0)).astype(np.float32)  # [77,3]
    sel77 = np.zeros((77, K), dtype=np.float32)
    sel77[r, r // 7] = 1.0                                    # [77,K] lhsT
    _CACHE["consts"] = dict(wm=_make_wm(), identb=identb, iotap=iotap,
                            iotaf128=iotaf128, sk113=sk113, m77k=m77k,
                            cvecs77=cvecs77, sel77=sel77)
    return _CACHE["consts"]


def make_in_maps(combined_preds, heatmaps, labels, masks):
    cpn = np.asarray(combined_preds, dtype=np.float32)
    hmn = np.asarray(heatmaps, dtype=np.float32)
    lbn = np.asarray(labels, dtype=np.float32)
    mkn = np.asarray(masks, dtype=np.float32)
    cc = _consts()
    in_maps = []
    for b in range(B):
        m = {
            "cp": np.ascontiguousarray(cpn[:, b]).reshape(S, C, P, F),
            "hm": np.ascontiguousarray(hmn[:, b]).reshape(S, K, P, F),
            "mk": np.ascontiguousarray(mkn[:, b, 0]).reshape(S, P, F),
            "lab": np.ascontiguousarray(lbn[b]),
        }
        m.update(cc)
        in_maps.append(m)
    return in_maps


def run_spmd(in_maps, trace=False, **kw):
    from concourse.bass_utils import run_bass_kernel_spmd
    return run_bass_kernel_spmd(get_nc(), in_maps, core_ids=list(range(B)),
                                trace=trace, **kw)


def kernel(combined_preds, heatmaps, labels, masks):
    res = run_spmd(make_in_maps(combined_preds, heatmaps, labels, masks)).results
    heat = np.stack([res[b]["out"][0, 0:3] for b in range(B)]).astype(np.float32)
    mask_l = np.stack([res[b]["out"][0, 3:6] for b in range(B)]).astype(np.float32)
    label = np.stack([res[b]["out"][0, 6:9] for b in range(B)]).astype(np.float32)
    return (heat, label, mask_l)
